# revision 1
# baseline (speedup 1.0000x reference)
"""GatedCrossAttention kernel for Trainium2 (8 NeuronCores).

Sharding: data-parallel over batch. B=8 == n_cores, so each core owns one
batch element end-to-end: all five matmuls, the norms/activations, and the
relu^2 attention run per-core with zero collectives; outputs are gathered
by the pmap. Shapes hardcoded per the problem spec:
  L=C=2048, B=8, E=1024, Z=256, MAXPOS=2048, f32.
"""

import math
from functools import partial

import jax
import jax.numpy as jnp
import numpy as np

E, Z, L, B, MAXPOS = 1024, 256, 2048, 8, 2048
EPS = 1e-5
_LEN_SCALE = 1.0 / math.sqrt(2048.0)


def _layernorm(x, w, b):
    mu = jnp.mean(x, axis=-1, keepdims=True)
    var = jnp.mean(jnp.square(x - mu), axis=-1, keepdims=True)
    return (x - mu) * jax.lax.rsqrt(var + EPS) * w + b


def _l2norm(x):
    n = jnp.sqrt(jnp.sum(jnp.square(x), axis=-1, keepdims=True))
    return x / jnp.maximum(n, EPS)


def _per_core(query, key_in, value, ln_w, ln_b, Wv, bv, Wk, bk, Wqru, bqru,
              Wh, bh, gamma, beta, bias):
    # query/key_in/value: [T, E] for this core's batch element; bias: [T, C]
    nq = _layernorm(query, ln_w, ln_b)
    g = gamma + 1.0
    base = nq @ Wqru.T + bqru                     # [T, 2E+Z]
    q, u, r = base[:, :Z], base[:, Z:Z + E], base[:, Z + E:]
    q = _l2norm(q) * g[0] + beta[0]               # [T, Z]
    u = jax.nn.sigmoid(u)
    r = jax.nn.silu(r)
    k = _l2norm(key_in @ Wk.T + bk) * g[1] + beta[1]   # [C, Z]
    v = jax.nn.silu(value @ Wv.T + bv)                 # [C, E]
    qk = q @ k.T * _LEN_SCALE + bias              # [T, C]
    attn = jnp.square(jax.nn.relu(qk))
    h = attn @ v                                   # [T, E]
    h = (h * r) @ Wh.T + bh
    return query + u * (h - query)


@partial(jax.pmap, axis_name="b",
         in_axes=(1, 1, 1) + (None,) * 13,
         out_axes=1)
def _pmapped(query, key_in, value, ln_w, ln_b, Wv, bv, Wk, bk, Wqru, bqru,
             Wh, bh, gamma, beta, bias):
    return _per_core(query, key_in, value, ln_w, ln_b, Wv, bv, Wk, bk,
                     Wqru, bqru, Wh, bh, gamma, beta, bias)


def kernel(query, key_in, value, ln_w, ln_b, Wv, bv, Wk, bk, Wqru, bqru,
           Wh, bh, gamma, beta, relpos):
    # Precompute the toeplitz rel-pos bias [L, C] on host (tiny, O(L*C)).
    relpos = np.asarray(relpos)
    idx = (np.arange(L)[None, :] - np.arange(L)[:, None]) + (MAXPOS - 1)
    bias = relpos[idx].astype(np.float32)          # [L, C]

    out = _pmapped(
        jnp.asarray(query), jnp.asarray(key_in), jnp.asarray(value),
        jnp.asarray(ln_w), jnp.asarray(ln_b), jnp.asarray(Wv),
        jnp.asarray(bv), jnp.asarray(Wk), jnp.asarray(bk),
        jnp.asarray(Wqru), jnp.asarray(bqru), jnp.asarray(Wh),
        jnp.asarray(bh), jnp.asarray(gamma), jnp.asarray(beta),
        jnp.asarray(bias),
    )
    return np.asarray(out).astype(np.float32)
